# revision 48
# baseline (speedup 1.0000x reference)
"""Causal self-attention (B=4, T=2048, C=2048, H=16, hd=128) on 8 trn2 cores.

Sharding: core = b*2 + half. Each core handles batch b and 8 heads
(half*8 .. half*8+7): tensor-parallel over heads within a batch, data
parallel over batch. Each core computes a partial out-projection
(contribution of its 8 heads); host sums the two partials per batch.

Device kernel v2 (all-SBUF, bf16 matmuls, software-pipelined):
  - x, q, k, v, y all stay SBUF-resident (no DRAM scratch round trips).
  - All matmul operands bf16 (fp32 PSUM accumulation); FWL fast weight
    loads. Rope applied on DVE from PSUM (even/odd dims pre-permuted
    into partition quadrant halves via host weight permutation).
  - Attention: transposed scores sT[k,q] per 512-q strip, causal
    trimmed at 128 granularity; exp on ACT (bf16 out); diagonal-block
    causal zeroing via gpsimd affine_select on the exp tile; attV +
    out-proj on PE. Softmax denominators: exp tiles accumulated on DVE,
    then ONE all-ones matmul per group fuses the cross-partition sum
    with its broadcast (gpsimd partition ops measured ~2us each on HW).
  - Per-head software pipeline: head h's attention emission is
    interleaved with head h+1's q/k projections; v computed per
    4-head quad; out-projection streams y from SBUF at the end.
"""

import zlib

import numpy as np
import ml_dtypes

import concourse.bass as bass
import concourse.tile as tile
from concourse import bacc, bass2jax, bass_isa, mybir

F32 = mybir.dt.float32
F32R = mybir.dt.float32r
BF16 = mybir.dt.bfloat16

B = 4
T = 2048
C = 2048
HD = 128
HL = 8          # local heads per core
NCC = 16        # contraction chunks of 128 over C
NTB = 16        # t blocks of 128
NQS = 4         # q strips of 512
SW = 512
N_CORES = 8

SWAP_MASK = list(range(16, 32)) + list(range(16))

# The neuronx-cc NEFF cache keys on the jit-level HLO (parameter
# shapes/dtypes), NOT the bass instruction stream. Encode a hash of this
# file + reps into the shape of a dummy parameter so any kernel change
# produces a fresh HLO hash (otherwise a stale NEFF gets reused).
try:
    with open(__file__, "rb") as _f:
        _CRC = zlib.crc32(_f.read())
except OSError:
    _CRC = 0


_PROBE_SKIP = set()


def _ckey_shape(reps):
    crc = _CRC ^ (0x9E3779B9 * reps)
    for s in sorted(_PROBE_SKIP):
        crc = zlib.crc32(s.encode(), crc)
    return [1 + crc % 251, 1 + (crc >> 8) % 251]


def build_program(reps=1, probe_skip=()):
    # probe_skip: timing-only probes that elide ops (breaks numerics!):
    #   "reduce" -> skip partition_all_reduce/partition_broadcast
    #   "select" -> skip affine_select diag masking
    #   "exp"    -> activation Copy instead of Exp
    global _PROBE_SKIP
    _PROBE_SKIP = set(probe_skip)
    nc = bacc.Bacc(None, target_bir_lowering=False)

    xT = nc.declare_dram_parameter("xT", [NCC, 128, T], BF16, isOutput=False)
    wq = nc.declare_dram_parameter("wq", [HL, 128, C], BF16, isOutput=False)
    wk = nc.declare_dram_parameter("wk", [HL, 128, C], BF16, isOutput=False)
    wv = nc.declare_dram_parameter("wv", [2, 128, NCC * SW], BF16, isOutput=False)
    wp = nc.declare_dram_parameter("wp", [HL, 128, C], BF16, isOutput=False)
    cs = nc.declare_dram_parameter("cs", [128, T], BF16, isOutput=False)
    ss = nc.declare_dram_parameter("ss", [128, T], BF16, isOutput=False)
    nc.declare_dram_parameter("ckey", _ckey_shape(reps), F32, isOutput=False)
    ones_in = nc.declare_dram_parameter("ones_in", [128, 128], F32R, isOutput=False)
    out = nc.declare_dram_parameter("out", [T, C], F32, isOutput=True)

    with tile.TileContext(nc) as tc:
        with (
            tc.tile_pool(name="const", bufs=1) as cpool,
            tc.tile_pool(name="vs", bufs=2) as vspool,
            tc.tile_pool(name="qk", bufs=2) as qkpool,
            tc.tile_pool(name="y", bufs=1) as ypool,
            tc.tile_pool(name="esb", bufs=3) as epool,
            tc.tile_pool(name="den", bufs=2) as dpool,
            tc.tile_pool(name="acc", bufs=3, space="PSUM") as accpool,
            tc.tile_pool(name="st", bufs=3, space="PSUM") as stpool,
            tc.tile_pool(name="pop", bufs=2, space="PSUM") as popool,
        ):
            cs_sb = cpool.tile([128, T], BF16, name="cs_sb", tag="cs")
            ss_sb = cpool.tile([128, T], BF16, name="ss_sb", tag="ss")
            # all-ones weights: one matmul fuses the softmax-denominator
            # cross-partition reduce AND its broadcast to 128 partitions
            ones128 = cpool.tile([128, 128], F32R, name="ones128", tag="ones")
            nc.sync.dma_start(out=ones128[:], in_=ones_in[:])

            for _rep in range(reps):
                emit_one_pass(
                    nc, tc, cs_sb, ss_sb, ones128,
                    xT, wq, wk, wv, wp, cs, ss, out,
                    ypool, vspool, qkpool, epool, dpool,
                    accpool, stpool, popool,
                )

    nc.compile()
    return nc


def emit_one_pass(
    nc, tc, cs_sb, ss_sb, ones128,
    xT, wq, wk, wv, wp, cs, ss, out,
    ypool, vspool, qkpool, epool, dpool,
    accpool, stpool, popool,
):
    import concourse.tile as tile  # noqa: F401

    if True:
        if True:
            ysb = [
                ypool.tile([128, T], BF16, name=f"ysb{h}", tag=f"y{h}")
                for h in range(HL)
            ]

            vq_tiles = [None, None]
            qt = {}
            kt = {}

            with (
                tc.tile_pool(name="xin", bufs=1) as xpool,
                tc.tile_pool(name="wvp", bufs=1) as wvpool,
                tc.tile_pool(name="wqk", bufs=4) as wqkpool,
                tc.tile_pool(name="rope", bufs=2) as rpool,
            ):
                def emit_qk_weights(h):
                    wt = wqkpool.tile([128, C], BF16, name=f"wq{h}", tag="wqk")
                    nc.sync.dma_start(out=wt[:], in_=wq[h])
                    qt[h] = (
                        wt,
                        qkpool.tile([128, T], BF16, name=f"qt{h}", tag="q"),
                    )
                    wt = wqkpool.tile([128, C], BF16, name=f"wk{h}", tag="wqk")
                    nc.sync.dma_start(out=wt[:], in_=wk[h])
                    kt[h] = (
                        wt,
                        qkpool.tile([128, T], BF16, name=f"kt{h}", tag="k"),
                    )

                # prologue DMA order: head-0 weights + wv0 + rope tables
                # first (small), then x — so PE/v work streams behind the
                # x tiles as they arrive.
                emit_qk_weights(0)
                wv0sb = wvpool.tile([128, NCC * SW], BF16, name="wvsb0", tag="wv")
                nc.sync.dma_start(out=wv0sb[:], in_=wv[0])
                nc.sync.dma_start(out=cs_sb[:], in_=cs[:])
                nc.sync.dma_start(out=ss_sb[:], in_=ss[:])

                xsb = []
                for cc in range(NCC):
                    xt = xpool.tile([128, T], BF16, name=f"xsb{cc}", tag=f"x{cc}")
                    nc.sync.dma_start(out=xt[:], in_=xT[cc])
                    xsb.append(xt)

                def emit_v_quad(qd, pool, ptag):
                    if qd == 0:
                        wvsb = wv0sb
                    else:
                        wvsb = wvpool.tile(
                            [128, NCC * SW], BF16, name=f"wvsb{qd}", tag="wv"
                        )
                        nc.sync.dma_start(out=wvsb[:], in_=wv[qd])
                    vqt = vspool.tile(
                        [128, NTB, SW], BF16, name=f"vs{qd}", tag="vs"
                    )
                    for tb in range(NTB):
                        pv = pool.tile([128, SW], F32, name="pv", tag=ptag)
                        for ccq in range(NCC):
                            nc.tensor.matmul(
                                pv[:],
                                xsb[ccq][:, tb * 128 : (tb + 1) * 128],
                                wvsb[:, ccq * SW : (ccq + 1) * SW],
                                start=(ccq == 0),
                                stop=(ccq == NCC - 1),
                            )
                        nc.scalar.copy(vqt[:, tb, :], pv[:])
                    vq_tiles[qd] = vqt

                def emit_proj_strip(h, j):
                    # j 0..7: 0-3 are q strips, 4-7 are k strips of head h
                    wsb, dst = qt[h] if j < 4 else kt[h]
                    s = j % 4
                    sl = slice(s * SW, (s + 1) * SW)
                    ps = accpool.tile([128, SW], F32, name="pp", tag="acc")
                    for ccq in range(NCC):
                        nc.tensor.matmul(
                            ps[:],
                            wsb[:, ccq * 128 : (ccq + 1) * 128],
                            xsb[ccq][:, sl],
                            start=(ccq == 0),
                            stop=(ccq == NCC - 1),
                        )
                    # drain PSUM on ACT; rope in bf16 on DVE (2x rate)
                    qb = rpool.tile([128, SW], BF16, name="qb", tag="qb")
                    nc.scalar.copy(qb[:], ps[:])
                    if "rope" in _PROBE_SKIP:
                        nc.vector.tensor_copy(dst[:, sl], qb[:])
                        return
                    t1 = rpool.tile([128, SW], BF16, name="t1", tag="t1")
                    nc.vector.tensor_mul(t1[:], qb[:], cs_sb[:, sl])
                    qsw = rpool.tile([128, SW], BF16, name="qsw", tag="qsw")
                    nc.vector.stream_shuffle(qsw[:], qb[:], SWAP_MASK)
                    t2 = rpool.tile([128, SW], BF16, name="t2", tag="t2")
                    nc.vector.tensor_mul(t2[:], qsw[:], ss_sb[:, sl])
                    with nc.allow_low_precision(reason="rope rotation in bf16"):
                        nc.vector.tensor_add(dst[:, sl], t1[:], t2[:])

                def emit_attn_group(h, g):
                    qd, r = h // 4, h % 4
                    vqt = vq_tiles[qd]
                    qsb = qt[h][1]
                    ksb = kt[h][1]
                    po = popool.tile([128, SW], F32, name="po", tag="po")
                    S = dpool.tile([128, SW], F32R, name="S", tag="S")
                    nkb = 4 * g + 4
                    for kb in range(nkb):
                        off = 128 * (kb - 4 * g) if kb >= 4 * g else 0
                        pst = stpool.tile([128, SW], F32, name="pst", tag="st")
                        nc.tensor.matmul(
                            pst[:, off:SW],
                            ksb[:, kb * 128 : (kb + 1) * 128],
                            qsb[:, g * SW + off : (g + 1) * SW],
                            start=True,
                            stop=True,
                        )
                        es = epool.tile([128, SW], BF16, name="es", tag="es")
                        nc.scalar.activation(
                            es[:, off:SW],
                            pst[:, off:SW],
                            mybir.ActivationFunctionType.Copy
                            if "exp" in _PROBE_SKIP
                            else mybir.ActivationFunctionType.Exp,
                        )
                        if kb >= 4 * g and "select" not in _PROBE_SKIP:
                            # zero exp where q < k inside the diagonal block
                            nc.gpsimd.affine_select(
                                out=es[:, off : off + 128],
                                in_=es[:, off : off + 128],
                                compare_op=mybir.AluOpType.is_ge,
                                fill=0.0,
                                base=0,
                                pattern=[[1, 128]],
                                channel_multiplier=-1,
                            )
                        nc.tensor.matmul(
                            po[:, off:SW],
                            vqt[:, kb, r * 128 : (r + 1) * 128],
                            es[:, off:SW],
                            start=(kb == 0),
                            stop=(kb == nkb - 1),
                        )
                        if kb == 0:
                            nc.vector.tensor_copy(S[:], es[:])
                        elif "S" not in _PROBE_SKIP:
                            nc.vector.tensor_add(
                                S[:, off:], S[:, off:], es[:, off:]
                            )
                    # one N=512 matmul: colsum over partitions, replicated
                    # to all 128 output partitions by the all-ones weights
                    csum = stpool.tile([128, SW], F32, name="csum", tag="st")
                    nc.tensor.matmul(
                        csum[:], ones128[:], S[:], start=True, stop=True
                    )
                    rcp = dpool.tile([128, SW], BF16, name="rcp", tag="rcp")
                    with nc.allow_low_precision(reason="softmax denom in bf16"):
                        nc.vector.reciprocal(rcp[:], csum[:])
                    nc.vector.tensor_mul(
                        ysb[h][:, g * SW : (g + 1) * SW], po[:], rcp[:]
                    )

                for j in range(8):
                    emit_proj_strip(0, j)
                emit_v_quad(0, stpool, "st")
                for h in range(HL):
                    for g in range(NQS):
                        emit_attn_group(h, g)
                        if h < HL - 1:
                            if g == 0:
                                emit_qk_weights(h + 1)
                            emit_proj_strip(h + 1, 2 * g)
                            emit_proj_strip(h + 1, 2 * g + 1)
                    if h == 1:
                        emit_v_quad(1, accpool, "acc")

            # ---------------- out projection ----------------
            with (
                tc.tile_pool(name="wpp", bufs=1) as wppool,
                tc.tile_pool(name="osb", bufs=3) as ospool,
            ):
                wpsb = []
                for cb in range(HL):
                    wt = wppool.tile([128, C], BF16, name=f"wp{cb}", tag=f"wp{cb}")
                    nc.sync.dma_start(out=wt[:], in_=wp[cb])
                    wpsb.append(wt)
                for tb in range(NTB):
                    for csi in range(4):
                        pf = accpool.tile([128, SW], F32, name="pf", tag="acc")
                        for cb in range(HL):
                            nc.tensor.matmul(
                                pf[:],
                                ysb[cb][:, tb * 128 : (tb + 1) * 128],
                                wpsb[cb][:, csi * SW : (csi + 1) * SW],
                                start=(cb == 0),
                                stop=(cb == HL - 1),
                            )
                        osb = ospool.tile([128, SW], F32, name="osb", tag="osb")
                        nc.scalar.copy(osb[:], pf[:])
                        nc.sync.dma_start(
                            out=out[
                                tb * 128 : (tb + 1) * 128,
                                csi * SW : (csi + 1) * SW,
                            ],
                            in_=osb[:],
                        )


# Per-head permutation of the 128 head dims: quadrant qd (32 partitions)
# holds rope pairs p = qd*16..qd*16+15 — even dims (2p) in slots 0..15,
# odd dims (2p+1) in slots 16..31. The rope partner swap is then a
# within-quadrant stream_shuffle by +-16.
_PERM = np.concatenate(
    [
        np.concatenate([2 * (qd * 16 + np.arange(16)) + r for r in (0, 1)])
        for qd in range(4)
    ]
)
# pair index held by each partition slot
_PAIR_OF_SLOT = np.concatenate(
    [np.tile(qd * 16 + np.arange(16), 2) for qd in range(4)]
)
# +1 on odd slots, -1 on even slots (sign of the sin term)
_SIN_SIGN = np.concatenate([np.repeat([-1.0, 1.0], 16) for _ in range(4)])

_BF = ml_dtypes.bfloat16


def prepare_core_inputs(x, Wq, Wk, Wv, Wp):
    """Returns list of 8 input dicts, core = b*2 + half."""
    scale = 1.0 / np.sqrt(HD)

    inv_freq = (1.0 / (10000.0 ** (np.arange(0, HD, 2) / HD))).astype(np.float64)
    freqs = np.outer(inv_freq[_PAIR_OF_SLOT], np.arange(T, dtype=np.float64))
    cs = np.cos(freqs).astype(_BF)
    ss = (np.sin(freqs) * _SIN_SIGN[:, None]).astype(_BF)

    halves = []
    for half in range(2):
        r0 = half * HL * HD  # first global row of this half's heads
        wq_in = np.empty((HL, 128, C), _BF)
        wk_in = np.empty((HL, 128, C), _BF)
        for h in range(HL):
            for arr, W, sc in ((wq_in, Wq, scale), (wk_in, Wk, 1.0)):
                Wh = W[r0 + h * HD : r0 + (h + 1) * HD][_PERM] * sc  # [128 d, C]
                # arr[h, p, cc*128+d] = Wh[d, cc*128+p]
                arr[h] = np.ascontiguousarray(
                    Wh.reshape(128, NCC, 128).transpose(2, 1, 0).reshape(128, C)
                ).astype(_BF)
        Wv_half = Wv[r0 : r0 + HL * HD]  # [1024, C]
        wv_in = np.empty((2, 128, NCC * SW), _BF)
        for qd in range(2):
            Wv4 = Wv_half[qd * SW : (qd + 1) * SW]  # [512 d4, C]
            wv_in[qd] = (
                Wv4.reshape(SW, NCC, 128).transpose(2, 1, 0).reshape(128, NCC * SW)
            ).astype(_BF)
        wp_in = np.ascontiguousarray(
            Wp.T[r0 : r0 + HL * HD].reshape(HL, 128, C)
        ).astype(_BF)
        halves.append((wq_in, wk_in, wv_in, wp_in))

    in_maps = []
    for b in range(B):
        xTb = np.ascontiguousarray(x[b].T).reshape(NCC, 128, T).astype(_BF)
        for half in range(2):
            wq_in, wk_in, wv_in, wp_in = halves[half]
            in_maps.append(
                {
                    "xT": xTb,
                    "wq": wq_in,
                    "wk": wk_in,
                    "wv": wv_in,
                    "wp": wp_in,
                    "cs": cs,
                    "ss": ss,
                    "ones_in": np.ones((128, 128), np.float32),
                }
            )
    return in_maps


_RUNNER_CACHE = None


class _Runner:
    def __init__(self, sharded, mesh, in_names, out_names, out_avals, zero_shapes):
        self.sharded = sharded
        self.mesh = mesh
        self.in_names = in_names
        self.out_names = out_names
        self.out_avals = out_avals
        self.zero_shapes = zero_shapes
        self.body = None       # set by _make_runner
        self.n_params = None
        self.donate = None

    def make_chained(self, n):
        """jit that runs the kernel n times back-to-back per dispatch,
        serialized by threading outputs into the next call's donated
        output operands."""
        import jax
        from jax.experimental.shard_map import shard_map
        from jax.sharding import PartitionSpec

        body, n_params = self.body, self.n_params

        def chained(*args):
            ins = args[:n_params]
            outs = tuple(args[n_params:])
            for _ in range(n):
                outs = body(*ins, *outs)
            return outs

        n_outs = len(self.out_names)
        in_specs = (PartitionSpec("core"),) * (n_params + n_outs)
        out_specs = (PartitionSpec("core"),) * n_outs
        return jax.jit(
            shard_map(
                chained,
                mesh=self.mesh,
                in_specs=in_specs,
                out_specs=out_specs,
                check_rep=False,
            ),
            donate_argnums=self.donate,
            keep_unused=True,
        )

    def concat_inputs(self, in_maps):
        arrs = []
        for i, name in enumerate(self.in_names):
            if name == "ckey":
                shape, dtype = self.in_shapes[i]
                arrs.append(np.zeros((N_CORES * shape[0], *shape[1:]), dtype))
            else:
                arrs.append(
                    np.concatenate([np.asarray(m[name]) for m in in_maps], axis=0)
                )
        return arrs

    def make_zeros(self):
        return [np.zeros((N_CORES * s[0], *s[1:]), d) for (s, d) in self.zero_shapes]

    def run(self, in_maps):
        out_arrs = self.sharded(*self.concat_inputs(in_maps), *self.make_zeros())
        return [
            {
                name: np.asarray(out_arrs[i]).reshape(
                    N_CORES, *self.out_avals[i].shape
                )[c]
                for i, name in enumerate(self.out_names)
            }
            for c in range(N_CORES)
        ]


def _make_runner(nc=None):
    """Compile the Bass program once and return a _Runner that reuses the
    jitted executable across calls. Mirrors bass2jax.run_bass_via_pjrt's
    multi-core branch."""
    import jax
    from jax.experimental.shard_map import shard_map
    from jax.sharding import Mesh, PartitionSpec

    if nc is None:
        nc = build_program()
    bass2jax.install_neuronx_cc_hook()

    partition_name = nc.partition_id_tensor.name if nc.partition_id_tensor else None
    in_names, in_shapes, out_names, out_avals, zero_shapes = [], [], [], [], []
    for alloc in nc.m.functions[0].allocations:
        if not isinstance(alloc, mybir.MemoryLocationSet):
            continue
        name = alloc.memorylocations[0].name
        if alloc.kind == "ExternalInput":
            if name != partition_name:
                in_names.append(name)
                in_shapes.append(
                    (tuple(alloc.tensor_shape), mybir.dt.np(alloc.dtype))
                )
        elif alloc.kind == "ExternalOutput":
            shape = tuple(alloc.tensor_shape)
            dtype = mybir.dt.np(alloc.dtype)
            out_names.append(name)
            out_avals.append(jax.core.ShapedArray(shape, dtype))
            zero_shapes.append((shape, dtype))
    n_params = len(in_names)
    n_outs = len(out_avals)
    all_in_names = list(in_names) + list(out_names)
    if partition_name is not None:
        all_in_names.append(partition_name)
    donate = tuple(range(n_params, n_params + n_outs))

    def _body(*args):
        operands = list(args)
        if partition_name is not None:
            operands.append(bass2jax.partition_id_tensor())
        outs = bass2jax._bass_exec_p.bind(
            *operands,
            out_avals=tuple(out_avals),
            in_names=tuple(all_in_names),
            out_names=tuple(out_names),
            lowering_input_output_aliases=(),
            sim_require_finite=True,
            sim_require_nnan=True,
            nc=nc,
        )
        return tuple(outs)

    devices = jax.devices()[:N_CORES]
    mesh = Mesh(np.asarray(devices), ("core",))
    in_specs = (PartitionSpec("core"),) * (n_params + n_outs)
    out_specs = (PartitionSpec("core"),) * n_outs
    sharded = jax.jit(
        shard_map(
            _body, mesh=mesh, in_specs=in_specs, out_specs=out_specs, check_rep=False
        ),
        donate_argnums=donate,
        keep_unused=True,
    )
    r = _Runner(sharded, mesh, in_names, out_names, out_avals, zero_shapes)
    r.body = _body
    r.n_params = n_params
    r.donate = donate
    r.in_shapes = in_shapes
    return r


def get_runner():
    global _RUNNER_CACHE
    if _RUNNER_CACHE is None:
        _RUNNER_CACHE = _make_runner()
    return _RUNNER_CACHE


def kernel(x, Wq, Wk, Wv, Wp):
    runner = get_runner()
    in_maps = prepare_core_inputs(
        np.asarray(x), np.asarray(Wq), np.asarray(Wk), np.asarray(Wv), np.asarray(Wp)
    )
    res = runner.run(in_maps)
    out = np.empty((B, T, C), np.float32)
    for b in range(B):
        np.add(res[2 * b]["out"], res[2 * b + 1]["out"], out=out[b])
    return out
